# revision 44
# baseline (speedup 1.0000x reference)
"""BatchHardTripletLoss on 8 trn2 NeuronCores (Bass/Tile, SPMD data-parallel).

v5 strategy:

  * Host packs whole label-groups into 8 bins of exactly 512 rows
    (subset-sum DP), sorts rows bin-by-bin, and rotates each core's column
    order so the core's OWN bin is column chunk 0.  Consequence: every
    anchor's positives (and self) live entirely in chunk 0, so
      - the hardest-positive reduce scans ONLY chunk 0 (1/8 the work),
      - the -4 "shift" mask matmul is needed ONLY for chunk 0, with a
        compact per-core label one-hot (<=128 classes -> contraction 128).
  * Embeddings ship as RAW bf16 (half the HBM traffic of fp32) and are
    never normalized on device.  The kernel computes the raw gram
      G[i,j] = x_i . x_j   (bf16 inputs, fp32 PSUM accumulate)
    and fuses the column normalization INTO the reduce with a custom DVE
    op (ANT_TTR_MAX, registered below):
      accum[i] = max(c0, max_j (G[i,j] * rinv[j] * c1))
    One op, two uses: c1=+1 gives the hardest-negative running max;
    c1=-1 gives -(hardest-positive min).  The row factor rinv[i] > 0
    commutes with min/max and is applied to the [128,1] outputs at the
    end.  This removes the whole 2M-element normalization pass the DVE
    would otherwise run, so the DVE (reduces) stays strictly faster than
    the PE (matmuls) and the PE HAM clock-gate holds 2.4 GHz.
  * Shift correctness with raw gram: chunk-0 psum gets
      S[i,j] = -4 * r_i * r_j * [label_i == label_j]
    via one compact one-hot matmul; the +-2 fp8 one-hots ship from host
    and are scaled by r (= column norms, bf16) on device.  After *rinv_j
    the shifted value is r_i*(sim - 4) -- strictly below every negative
    r_i*sim.  Per anchor:
      loss = relu( rinv_i*(maxacc + negminacc) - 4 + margin ) * valid
  * Norms: ACT/DVE squares + k-major ones-matmul column sums (all four
    chunk-sums of a half share one PSUM bank at 32-aligned partitions),
    Newton rsqrt on [128,16]-gathered tiles, two halves pipelined with
    the half-row (4KB-line, ~216 GB/s) DMA stream.
  * Tiny warm-up matmuls run during the DMA phase (and fill colsum
    gaps) so the PE HAM clock-gate reaches 2.4 GHz before the main
    matmul stream; ACT sqrt/relu tables are preloaded with dummy ops.

Cross-core reduction: each core returns NM partial sums; host adds and
divides by n_valid (labels-only, host-computed).
"""

import os
from contextlib import ExitStack

import numpy as np
import ml_dtypes

import concourse.bass as bass
import concourse.bacc as bacc
import concourse.mybir as mybir
import concourse.tile as tile
from concourse.bass_utils import run_bass_kernel_spmd

F32 = mybir.dt.float32
F32R = mybir.dt.float32r
BF16 = mybir.dt.bfloat16
FP8 = mybir.dt.float8e4
AF = mybir.ActivationFunctionType
ALU = mybir.AluOpType
AX = mybir.AxisListType

B, D, C = 4096, 512, 512
NCORES = 8
RPC = B // NCORES            # rows per core = 512
NCH = 512                    # column chunk size (PSUM bank = 512 fp32)
CCMP = 128                   # compact one-hot size (distinct labels per bin)
MARGIN = 0.2
BIG = 4.0
NEG_INIT = -60000.0
N_WARMUP = 24
N_WARMFILL = 8


# ---- custom DVE op: accum = max(C0, max_k(Src0*Src1*C1)) ----------------
def _register_ttr_max():
    from concourse import dve_ops as _dvo
    from concourse.dve_spec import (Spec, Src0, Src1, C0, C1, lower, maxx,
                                    _has_src1)
    from concourse.dve_uop import DveOpSpec

    name = "ANT_TTR_MAX"
    if name in _dvo._SUB_OPCODE_FOR_NAME:
        return next(op for op in _dvo.OPS if op.name == name)

    def _ref(in0, in1, c0, c1, c2):
        b = (in0.astype(np.float32) * in1 * c1).astype(np.float32)
        seed = np.asarray(c0, np.float32)
        seed = seed.reshape(-1, 1) if seed.ndim else seed
        red = np.maximum(b.reshape(b.shape[0], -1).max(axis=-1, keepdims=True),
                         seed)
        return b, red

    spec = Spec(body=Src0 * Src1 * C1, accum=maxx, accum_init=C0,
                reference=_ref)
    row = _dvo._CUSTOM_DVE_ROW_BASE + len(_dvo.OPS)
    assert row < 0x20, row
    _dvo._SUB_OPCODE_FOR_NAME[name] = row
    shas = {}
    for ver in ("v3", "v4"):
        try:
            s = DveOpSpec(name=name, opcode=row, uops=lower(spec, ver=ver),
                          rd1_en=_has_src1(spec))
            shas[ver] = s.sha(ver)
        except Exception:
            pass
    op = _dvo.DveOp(name, spec, subdim=False, uops_sha=shas)
    _dvo.OPS.append(op)
    _dvo.CUSTOM_DVE_SPECS[name] = spec
    return op


TTR_MAX = _register_ttr_max()


def build_program(Bf=B, Df=D, rpc=RPC):
    assert Df % 128 == 0 and Bf % NCH == 0 and rpc == NCH
    KD = Df // 128           # 4 contraction tiles
    NM = rpc // 128          # 4 row tiles per core
    NN = Bf // NCH           # 8 column chunks
    (cl0, cw0), (cl1, cw1) = (0, NN // 2), (NN // 2, NN // 2)

    nc = bacc.Bacc("TRN2", target_bir_lowering=False, debug=False)
    ET_d = nc.declare_dram_parameter("ET", [Df, Bf], BF16, isOutput=False)
    OTp_d = nc.declare_dram_parameter("OTp", [CCMP, rpc], FP8, isOutput=False)
    OTn_d = nc.declare_dram_parameter("OTn", [CCMP, rpc], FP8, isOutput=False)
    val_d = nc.declare_dram_parameter("valid", [128, NM], F32, isOutput=False)
    out_d = nc.declare_dram_parameter("out", [1, NM], F32, isOutput=True)

    with tile.TileContext(nc) as tc, ExitStack() as ctx:
        const = ctx.enter_context(tc.tile_pool(name="const", bufs=1))
        big = ctx.enter_context(tc.tile_pool(name="big", bufs=KD))
        sqp = ctx.enter_context(tc.tile_pool(name="sq", bufs=20))
        rbp = ctx.enter_context(tc.tile_pool(name="rb", bufs=5))
        smalls = ctx.enter_context(tc.tile_pool(name="small", bufs=1))
        psA = ctx.enter_context(tc.tile_pool(name="psA", bufs=2, space="PSUM"))
        psB = ctx.enter_context(tc.tile_pool(name="psB", bufs=1, space="PSUM"))
        psM = ctx.enter_context(tc.tile_pool(name="psM", bufs=6, space="PSUM"))

        # ---- constants --------------------------------------------------
        ones_cb = const.tile([128, 1], BF16, tag="ones_cb")
        nc.vector.memset(ones_cb[:], 1.0)
        ones_r_f = const.tile([1, 128], F32, tag="ones_r")
        nc.vector.memset(ones_r_f[:], 1.0)
        ones_r = ones_r_f[:].bitcast(F32R)
        ones_cf = const.tile([128, 1], F32, tag="ones_cf")
        nc.vector.memset(ones_cf[:], 1.0)
        relu_bias = const.tile([128, 1], F32, tag="relu_bias")
        nc.vector.memset(relu_bias[:], MARGIN - BIG)
        neg_init = const.tile([128, 1], F32, tag="neg_init")
        nc.vector.memset(neg_init[:], NEG_INIT)
        warm = const.tile([128, 64], BF16, tag="warm")
        nc.vector.memset(warm[:], 0.125)
        # dummy sqrt/relu: pull the ACT_TABLE_LOADs (~1.3us each) into the
        # DMA phase instead of the newton / loss critical paths
        tbl_scr = const.tile([1, 2], F32, tag="tbl_scr")
        nc.scalar.sqrt(tbl_scr[:], warm[0:1, 0:2])
        nc.scalar.activation(tbl_scr[:], warm[0:1, 0:2], AF.Relu)

        # ---- PE warm-up: keep HAM busy while DMAs land ------------------
        wps = psB.tile([64, 64], F32, tag="rb", name="warm_ps", bufs=1)

        def emit_warmfill(n_reps):
            for _ in range(n_reps):
                nc.tensor.matmul(wps[:], lhsT=warm[:, 0:64], rhs=warm[:],
                                 start=True, stop=True)

        emit_warmfill(N_WARMUP)

        # ---- DMA loads --------------------------------------------------
        val_t = const.tile([128, NM], F32, tag="val")
        nc.sync.dma_start(val_t[:], val_d[:, :])
        otp_f8 = smalls.tile([CCMP, rpc], FP8, tag="otp_f8")
        nc.sync.dma_start(otp_f8[:], OTp_d[:, :])
        otn_f8 = smalls.tile([CCMP, rpc], FP8, tag="otn_f8")
        nc.sync.dma_start(otn_f8[:], OTn_d[:, :])

        et_tiles = [
            big.tile([128, Bf], BF16, tag="big", name=f"et{k}") for k in range(KD)
        ]

        def emit_loads(cl, cw):
            # full-row pieces, 8KB partition lines (~247 GB/s measured)
            for k in range(KD):
                nc.sync.dma_start(
                    et_tiles[k][:, cl * NCH:(cl + cw) * NCH],
                    ET_d[k * 128:(k + 1) * 128, cl * NCH:(cl + cw) * NCH],
                )

        # ---- column sums of squares ------------------------------------
        row_buf = smalls.tile([1, Bf], F32, tag="rowbuf")       # ssq row
        r_row = smalls.tile([1, Bf], F32, tag="rrow")           # rinv row
        nrm_row = smalls.tile([1, NCH], F32, tag="nrmrow")      # r (chunk 0)

        def emit_colsums_half(h, sq_eng, fill=0):
            # k-major: chunk j's colsum accumulates as each k-piece lands.
            # All 4 chunk-sums share ONE psum bank at partitions 32j.
            cs4 = psA.tile([128, NCH], F32, tag="cs4", name="cs4", bufs=1)
            for k in range(KD):
                for j in range(4):
                    jj = 4 * h + j
                    sq = sqp.tile([128, NCH], BF16, tag="sq", name="sq")
                    srcap = et_tiles[k][:, bass.ts(jj, NCH)]
                    eng = sq_eng[(k * 4 + j) % len(sq_eng)]
                    if eng == "act":
                        nc.scalar.activation(sq[:], srcap, AF.Square)
                    elif eng == "dve":
                        nc.vector.tensor_tensor(sq[:], srcap, srcap, ALU.mult)
                    else:
                        nc.gpsimd.tensor_tensor(sq[:], srcap, srcap, ALU.mult)
                    nc.tensor.matmul(cs4[32 * j:32 * j + 1, :], lhsT=ones_cb[:],
                                     rhs=sq[:], start=(k == 0),
                                     stop=(k == KD - 1), skip_group_check=True,
                                     tile_position=(0, 32 * j))
                if fill:
                    emit_warmfill(fill)
            for j in range(4):
                nc.scalar.copy(row_buf[0:1, bass.ts(4 * h + j, NCH)],
                               cs4[32 * j:32 * j + 1, :])

        # ---- Newton rsqrt on [128, 16] gathered tiles -------------------
        def emit_rsqrt(h, cl, cw):
            fd = cw * NCH // 128
            ssq = smalls.tile([128, fd], F32, tag=f"ssq{h}", name=f"ssq{h}")
            nc.gpsimd.dma_start(ssq[:, :], row_buf[0:1, cl * NCH:(cl + cw) * NCH])
            nrm = smalls.tile([128, fd], F32, tag=f"nrm{h}", name=f"nrm{h}")
            nc.scalar.sqrt(nrm[:], ssq[:])
            r0 = smalls.tile([128, fd], F32, tag=f"r0{h}", name=f"r0{h}")
            nc.vector.reciprocal_approx_fast(r0[:], nrm[:])
            t1 = smalls.tile([128, fd], F32, tag=f"nt1{h}", name=f"nt1{h}")
            nc.vector.tensor_tensor(t1[:], r0[:], r0[:], ALU.mult)
            t2 = smalls.tile([128, fd], F32, tag=f"nt2{h}", name=f"nt2{h}")
            nc.vector.tensor_tensor(t2[:], t1[:], ssq[:], ALU.mult)
            nc.vector.tensor_scalar(t2[:], t2[:], -0.5, 1.5, ALU.mult, ALU.add)
            r8 = smalls.tile([128, fd], F32, tag=f"r8{h}", name=f"r8{h}")
            nc.vector.tensor_tensor(r8[:], r0[:], t2[:], ALU.mult)
            nc.gpsimd.dma_start(r_row[0:1, cl * NCH:(cl + cw) * NCH], r8[:, :])
            if cl == 0:
                # norms of chunk 0 (= partitions [0, 512//fd) of this tile)
                nc.gpsimd.dma_start(nrm_row[0:1, :], nrm[0:512 // fd, :])

        # ---- device-scaled compact one-hots (shift = -4 r_i r_j) --------
        otp_s = smalls.tile([CCMP, rpc], BF16, tag="otp_s")
        otn_s = smalls.tile([CCMP, rpc], BF16, tag="otn_s")

        def emit_ot_scale():
            rbn_ps = psB.tile([128, NCH], F32, tag="rb", name="rbn")
            nc.tensor.matmul(rbn_ps[:], lhsT=ones_r,
                             rhs=nrm_row[0:1, :].bitcast(F32R),
                             start=True, stop=True)
            rbn = rbp.tile([128, NCH], BF16, tag="rbn_sb", name="rbn_sb", bufs=1)
            nc.vector.tensor_copy(rbn[:], rbn_ps[:])
            otp_b = smalls.tile([CCMP, rpc], BF16, tag="otp_b")
            nc.scalar.copy(otp_b[:], otp_f8[:])
            otn_b = smalls.tile([CCMP, rpc], BF16, tag="otn_b")
            nc.scalar.copy(otn_b[:], otn_f8[:])
            nc.vector.tensor_tensor(otp_s[:], otp_b[:], rbn[0:CCMP, :], ALU.mult)
            nc.vector.tensor_tensor(otn_s[:], otn_b[:], rbn[0:CCMP, :], ALU.mult)

        # ---- rinv broadcast for the fused reduces -----------------------
        def emit_rb(n):
            rb_ps = psB.tile([128, NCH], F32, tag="rb", name="rb")
            nc.tensor.matmul(rb_ps[:], lhsT=ones_r,
                             rhs=r_row[0:1, bass.ts(n, NCH)].bitcast(F32R),
                             start=True, stop=True)
            rb = rbp.tile([128, NCH], F32, tag="rb_sb", name="rb_sb")
            nc.scalar.copy(rb[:], rb_ps[:])
            return rb

        # ---- main loop --------------------------------------------------
        mxs = [
            smalls.tile([128, NN], F32, tag=f"mx{m}", name=f"mx{m}")
            for m in range(NM)
        ]
        mns = [
            smalls.tile([128, 1], F32, tag=f"mn{m}", name=f"mn{m}")
            for m in range(NM)
        ]
        ttr_scr = smalls.tile([128, NCH], BF16, tag="ttr_scr")
        rinvcol = smalls.tile([128, NM], F32, tag="rinvcol")
        loss_all = smalls.tile([128, NM], F32, tag="lossall")

        def emit_final_m(m):
            hnm = smalls.tile([128, 1], F32, tag=f"hnm{m}", name=f"hnm{m}")
            nc.vector.tensor_reduce(hnm[:], mxs[m][:, :], AX.X, ALU.max)
            dlt = smalls.tile([128, 1], F32, tag=f"dlt{m}", name=f"dlt{m}")
            nc.vector.tensor_tensor(dlt[:], hnm[:], mns[m][:], ALU.add)
            sc = smalls.tile([128, 1], F32, tag=f"sc{m}", name=f"sc{m}")
            nc.vector.tensor_tensor(sc[:], dlt[:], rinvcol[:, m:m + 1], ALU.mult)
            rl = smalls.tile([128, 1], F32, tag=f"rl{m}", name=f"rl{m}")
            nc.scalar.activation(rl[:], sc[:], AF.Relu, bias=relu_bias[:])
            nc.vector.tensor_tensor(
                loss_all[:, m:m + 1], rl[:], val_t[:, m:m + 1], ALU.mult
            )

        def emit_blocks0_e(ms):
            tiles = []
            for m in ms:
                ps = psM.tile([128, NCH], F32, tag="ps", name="ps")
                for k in range(KD):
                    nc.tensor.matmul(
                        ps[:],
                        lhsT=et_tiles[k][:, bass.ts(m, 128)],
                        rhs=et_tiles[k][:, bass.ts(0, NCH)],
                        start=(k == 0), stop=False,
                    )
                tiles.append(ps)
            return tiles

        def emit_blocks0_tail(ps_tiles, rb):
            for m in range(NM):
                ps = ps_tiles[m]
                nc.tensor.matmul(
                    ps[:],
                    lhsT=otp_s[:, bass.ts(m, 128)],
                    rhs=otn_s[:, :],
                    start=False, stop=True,
                )
                nc.vector._custom_dve(
                    TTR_MAX, out=ttr_scr[:], in0=ps[:], in1=rb[:],
                    s0=neg_init[:], s1=1.0,
                    accum_out=mxs[m][:, 0:1],
                )
                nc.vector._custom_dve(
                    TTR_MAX, out=ttr_scr[:], in0=ps[:], in1=rb[:],
                    s0=neg_init[:], s1=-1.0,
                    accum_out=mns[m][:],
                )

        def emit_blocks(n, rb):
            for m in range(NM):
                ps = psM.tile([128, NCH], F32, tag="ps", name="ps")
                for k in range(KD):
                    nc.tensor.matmul(
                        ps[:],
                        lhsT=et_tiles[k][:, bass.ts(m, 128)],
                        rhs=et_tiles[k][:, bass.ts(n, NCH)],
                        start=(k == 0), stop=(k == KD - 1),
                    )
                nc.vector._custom_dve(
                    TTR_MAX, out=ttr_scr[:], in0=ps[:], in1=rb[:],
                    s0=neg_init[:], s1=1.0,
                    accum_out=mxs[m][:, n:n + 1],
                )

        # ---- emission schedule ------------------------------------------
        SQ = ["dve", "act"]
        emit_loads(cl0, cw0)
        emit_loads(cl1, cw1)
        emit_colsums_half(0, SQ, fill=10)
        emit_warmfill(28)
        ps0 = emit_blocks0_e([0, 1, 2, 3])
        emit_warmfill(16)
        emit_rsqrt(0, cl0, cw0)
        emit_ot_scale()
        rbs = {}
        for n in range(4):
            rbs[n] = emit_rb(n)
        emit_blocks0_tail(ps0, rbs[0])
        emit_blocks(1, rbs[1])
        emit_colsums_half(1, SQ)
        emit_rsqrt(1, cl1, cw1)
        emit_blocks(2, rbs[2])
        rbs[4] = emit_rb(4)
        rbs[5] = emit_rb(5)
        emit_blocks(3, rbs[3])
        rbs[6] = emit_rb(6)
        rbs[7] = emit_rb(7)
        for n in range(4, NN):
            emit_blocks(n, rbs[n])

        # ---- per-anchor loss --------------------------------------------
        for m in range(NM):
            nc.gpsimd.dma_start(rinvcol[:, m:m + 1],
                                r_row[0:1, m * 128:(m + 1) * 128])
        for m in range(NM):
            emit_final_m(m)

        out_ps = psA.tile([1, NM], F32, tag="cs4", name="out_ps", bufs=1)
        nc.tensor.matmul(out_ps[:], lhsT=ones_cf[:], rhs=loss_all[:, :],
                         start=True, stop=True)
        out_sb = smalls.tile([1, NM], F32, tag="outsb")
        nc.vector.tensor_copy(out_sb[:], out_ps[:])
        nc.sync.dma_start(out_d[:, :], out_sb[:])

    nc.compile()
    return nc


# ======================== host side =====================================

def _pack_bins(labels, nbins=NCORES, cap=RPC):
    """Pack whole label-groups into nbins bins of exactly `cap` rows.
    Greedy large-first + subset-sum DP (numpy shift-or) per bin."""
    vals, counts = np.unique(labels, return_counts=True)
    items = sorted(zip(vals.tolist(), counts.tolist()), key=lambda t: -t[1])
    bins = []
    remaining = items
    for b in range(nbins - 1):
        sizes = np.array([s for _, s in remaining], dtype=np.int64)
        reach = np.zeros(cap + 1, dtype=bool)
        reach[0] = True
        used_at = np.full(cap + 1, -1, dtype=np.int64)
        for i, s in enumerate(sizes):
            newly = np.zeros_like(reach)
            newly[s:] = reach[: cap + 1 - s]
            newly &= ~reach
            if newly.any():
                used_at[newly] = i
                reach |= newly
        if not reach[cap]:
            raise RuntimeError(f"bin {b}: exact packing infeasible")
        chosen = set()
        t = cap
        while t > 0:
            i = int(used_at[t])
            assert i >= 0 and i not in chosen
            chosen.add(i)
            t -= int(sizes[i])
        bins.append([remaining[i][0] for i in chosen])
        remaining = [it for i, it in enumerate(remaining) if i not in chosen]
    assert sum(s for _, s in remaining) == cap
    bins.append([lab for lab, _ in remaining])
    return bins


def host_prepare(embeddings, labels):
    """Layout prep: label-group packing, sorted row order, per-core column
    rotation, bf16 cast, compact one-hots, validity.  No embedding math."""
    embeddings = np.asarray(embeddings, dtype=np.float32)
    labels = np.asarray(labels).astype(np.int64)
    NM = RPC // 128
    NN = B // NCH

    bins = _pack_bins(labels)
    row_order = np.concatenate(
        [np.where(labels == l)[0] for labs in bins for l in labs]
    )
    lab_s = labels[row_order]
    ET_s = np.ascontiguousarray(embeddings[row_order].T.astype(ml_dtypes.bfloat16))

    cnt = np.bincount(labels, minlength=C)[labels]
    valid_full = ((cnt >= 2) & (cnt <= B - 1)).astype(np.float32)
    valid_s = valid_full[row_order]

    in_maps = []
    for c in range(NCORES):
        rows = slice(c * RPC, (c + 1) * RPC)
        labs = bins[c]
        assert len(labs) <= CCMP, f"core {c}: {len(labs)} labels > {CCMP}"
        lut = {l: g for g, l in enumerate(labs)}
        cl = np.array([lut[l] for l in lab_s[rows]], dtype=np.int64)
        otp = np.zeros((CCMP, RPC), dtype=np.float32)
        otp[cl, np.arange(RPC)] = 2.0
        otn = -otp
        order = [(c + d) % NN for d in range(NN)]
        colperm = np.concatenate(
            [np.arange(j * NCH, (j + 1) * NCH) for j in order]
        )
        in_maps.append(
            {
                "ET": np.ascontiguousarray(ET_s[:, colperm]),
                "OTp": otp.astype(ml_dtypes.float8_e4m3),
                "OTn": otn.astype(ml_dtypes.float8_e4m3),
                "valid": np.ascontiguousarray(
                    valid_s[rows].reshape(NM, 128).T
                ),
            }
        )
    return in_maps, valid_full


_prog_cache = {}


def _get_program():
    key = (B, D, C, RPC)
    if key not in _prog_cache:
        _prog_cache[key] = build_program()
    return _prog_cache[key]


LAST_RESULT = None


def kernel(embeddings, labels):
    global LAST_RESULT
    in_maps, valid = host_prepare(embeddings, labels)
    nc = _get_program()
    trace = bool(int(os.environ.get("TRIPLET_TRACE", "0")))
    res = run_bass_kernel_spmd(nc, in_maps, list(range(NCORES)), trace=trace)
    LAST_RESULT = res
    loss_sum = float(sum(r["out"].astype(np.float64).sum() for r in res.results))
    n_valid = max(int(valid.sum()), 1)
    return np.array(loss_sum / n_valid, dtype=np.float32)
